# revision 34
# baseline (speedup 1.0000x reference)
"""Windowed (8x8) multi-head attention for Trainium2, data-parallel over 8 cores.

Reference computation (shapes hardcoded):
  x [32, 64, 64, 384] -> window into [2048, 64, 384] (8x8 windows, 64 tok each)
  qkv = xw @ w_qkv [384, 1152]; 12 heads x 32 dims; softmax(q k^T / sqrt(32)) @ v
  out = attn_out @ w_out [384, 384] + b_out; un-window -> [32, 64, 64, 384]

Sharding: batch across 8 cores (4 batches = 256 windows = 16384 tokens/core),
driven by ONE shard_map executable whose body unrolls R_LOOP full problem
executions inside a single NEFF (see N_CORES/R_LOOP comment below).

Kernel layout strategy (per core):
  - host pre-transposes x to channel-major xT [384, 16384] (bf16), pre-scales
    the q columns of w_qkv by 1/sqrt(32).
  - qk^T computed channel-major ([c_out, tok]) so per-(head, window) q/k slices
    are matmul operands directly (d on partitions).
  - sim^T = k q^T ([keys, queries]) via K=32 matmuls packed 8-way with
    tile_position (4 heads x 2 windows concurrently in the PE array).
  - exp on ScalarE (no max subtraction needed: |sim| <~ 6).
  - AV matmul: lhsT = exp^T [keys, queries], rhs = v_ext [keys, 33] where
    col 32 is ones -> computes unnormalized out AND the softmax denominator.
  - normalize token-major with a broadcast tensor_tensor on DVE.
  - PE-transpose A to channel-major, out-proj, add broadcast bias, DMA out.

The per-window-pair work is software-pipelined 4 deep so the tensor engine
never waits on ScalarE (exp, PSUM drains) or DVE (normalize): at step s the
PE runs projections+sim for pair s, AV for pair s-1, transposes for pair s-2
and the out-projection for pair s-3, while the other engines fill the gaps.
CoreSim cost model at scale: PE 98.7% busy, 12.4 us/tile; HW lands at
~15 us/tile (weight-load stalls on the small sim/AV matmuls + sustained-load
clock throttle account for the difference; restructuring sim/AV into
block-diagonal 2-window matmuls is provably neutral once weight-load time
is counted, so this shape is the practical floor for bf16).
"""

import numpy as np
import ml_dtypes
from contextlib import ExitStack

import concourse.bass as bass
import concourse.tile as tile
from concourse import mybir
from concourse.bass_utils import run_bass_kernel_spmd
from concourse.masks import make_identity

B, H, W, D = 32, 64, 64, 384
HEADS, DH = 12, 32
WSZ = 8
S = WSZ * WSZ  # 64 tokens per window
SCALE = DH ** -0.5
# 8 cores via ONE shard_map executable. Measured axon-tunnel behavior:
#  - any blocking sync (block_until_ready / device_put+ready / fetch) costs
#    a fixed ~83 ms WAN round trip, paid once per chain;
#  - chained dispatches of a single multi-device executable stream at
#    device rate (per-call overhead ~0 in steady state);
#  - per-core separate executables instead serialize ~0.29 ms/execute in
#    the relay, which is why the earlier 2-core split looked optimal.
# R_LOOP kernel executions are inlined back-to-back in ONE NEFF (chained
# via a dummy y_in alias so they keep a data dependence), amortizing the
# per-dispatch mapping/launch cost to ~zero; per-run time is then pure
# device execution (~430 us, CoreSim cost model agrees).
N_CORES = 8
R_LOOP = 16
TOK_TOTAL = B * H * W          # 131072
TOK_CORE = TOK_TOTAL // N_CORES
T_TILE = 512                   # tokens per outer tile (8 windows)
WPS = T_TILE // 128            # window-pair steps per tile

BF16 = mybir.dt.bfloat16
F32 = mybir.dt.float32


def build_kernel(nc: bass.Bass, n_tok: int, split_waits: bool = True,
                 limit_iters: int | None = None, alias_in: bool = False,
                 repeat: int = 1, qk_on_gpsimd: bool = False):
    """Emit the per-core program. Single input blob [D, n_tok + 1920] bf16:
    cols [0, n_tok) = xT channel-major; [n_tok, +1152) = w_qkv (q-prescaled);
    [+1152, +1536) = w_out; row 0 of cols [+1536, +1920) = b_out.
    Output: y [n_tok, D] bf16 token-major."""
    assert n_tok % T_TILE == 0
    C = n_tok + 3 * D + D + D
    xb = nc.dram_tensor("xb", [D, C], BF16, kind="ExternalInput").ap()
    xT = xb[:, 0:n_tok]
    w_qkv = xb[:, n_tok:n_tok + 3 * D]
    w_out = xb[:, n_tok + 3 * D:n_tok + 4 * D]
    bias_off = n_tok + 4 * D
    y = nc.dram_tensor("y", [n_tok, D], BF16, kind="ExternalOutput").ap()
    if alias_in:
        # dummy ExternalInput the NKI lowering can alias the output onto:
        # chained bass_exec calls inside one jit body thread y -> y_in so
        # they keep a data dependence (no CSE) and reuse one buffer.
        nc.dram_tensor("y_in", [n_tok, D], BF16, kind="ExternalInput")

    n_iters = n_tok // T_TILE
    if limit_iters is not None:
        n_iters = min(n_iters, limit_iters)
    NW = n_iters * WPS

    # AV loop order: head h is ready once exp on sim bank r=h%4 lands, so
    # visit heads bank-major.
    AV_ORDER = [m * 4 + r for r in range(4) for m in range(3)]

    with tile.TileContext(nc) as tc, ExitStack() as ctx:
        consts = ctx.enter_context(tc.tile_pool(name="consts", bufs=1))
        px = ctx.enter_context(tc.tile_pool(name="px", bufs=3))
        pqk = ctx.enter_context(tc.tile_pool(name="pqk", bufs=2))
        pv = ctx.enter_context(tc.tile_pool(name="pv", bufs=4))
        pexp = ctx.enter_context(tc.tile_pool(name="pexp", bufs=4))
        pr = ctx.enter_context(tc.tile_pool(name="pr", bufs=4))
        pa = ctx.enter_context(tc.tile_pool(name="pa", bufs=4))
        pat = ctx.enter_context(tc.tile_pool(name="pat", bufs=4))
        po = ctx.enter_context(tc.tile_pool(name="po", bufs=4))
        # PSUM: 8 banks. Concurrent row-tiled matmuls need distinct banks:
        # 4 sim banks (one per 32-row group), 2 av/transpose banks (one tag
        # ring), 2 projection banks (qk/v/out rotate).
        ps_proj = ctx.enter_context(tc.tile_pool(name="ps_proj", bufs=2, space="PSUM"))
        ps_sim = ctx.enter_context(tc.tile_pool(name="ps_sim", bufs=4, space="PSUM"))
        ps_av = ctx.enter_context(tc.tile_pool(name="ps_av", bufs=2, space="PSUM"))

        # weights: [128, kc, c_out]
        wq_sb = consts.tile([128, 3, 3 * D], BF16)
        for kc in range(3):
            nc.sync.dma_start(out=wq_sb[:, kc, :], in_=w_qkv[kc * 128:(kc + 1) * 128, :])
        wo_sb = consts.tile([128, 3, D], BF16)
        for kc in range(3):
            nc.sync.dma_start(out=wo_sb[:, kc, :], in_=w_out[kc * 128:(kc + 1) * 128, :])
        # bias broadcast to all 128 partitions (row 0 of the blob's tail)
        bias_sb = consts.tile([128, D], BF16)
        nc.sync.dma_start(
            out=bias_sb,
            in_=bass.AP(tensor=xb.tensor, offset=xb.offset + bias_off,
                        ap=[[0, 128], [1, D]]),
        )
        ident = consts.tile([128, 128], BF16)
        make_identity(nc, ident)

        xt_tiles: dict[int, bass.AP] = {}
        qk_tiles: dict[int, bass.AP] = {}
        st: dict[int, dict] = {}

        def load_xt(it):
            # gpsimd's DGE queue: keeps input loads off the sync queue so the
            # per-step y stores can't head-of-line block them.
            xt = px.tile([128, 3, T_TILE], BF16, name="xt")
            t0 = (it % n_iters) * T_TILE
            for kc in range(3):
                nc.gpsimd.dma_start(
                    out=xt[:, kc, :], in_=xT[kc * 128:(kc + 1) * 128, t0:t0 + T_TILE]
                )
            xt_tiles[it] = xt

        def qk_group(it, m):
            """q,k projection for one 128-channel output group, channel-major."""
            xt = xt_tiles[it]
            ps = ps_proj.tile([128, T_TILE], F32, tag="proj", name="psqk")
            for kc in range(3):
                nc.tensor.matmul(
                    ps,
                    lhsT=wq_sb[:, kc, m * 128:(m + 1) * 128],
                    rhs=xt[:, kc, :],
                    start=(kc == 0),
                    stop=(kc == 2),
                )
            if qk_on_gpsimd:
                # split each drain column-wise across ScalarE and DVE: on HW
                # (unlike the CoreSim cost model) DVE runs ~2x slower and was
                # the per-step straggler; 288/224 puts both engines just
                # under the PE per-step budget (whole-op splits can't: 3/3
                # leaves DVE at/above PE, 4/2 overloads Act)
                CA = 288
                nc.scalar.copy(qk_tiles[it][:, m, 0:CA], ps[:, 0:CA])
                nc.vector.tensor_copy(
                    qk_tiles[it][:, m, CA:T_TILE], ps[:, CA:T_TILE])
            else:
                nc.vector.tensor_copy(qk_tiles[it][:, m, :], ps)

        def s1(s):
            """v projection + sim^T + exp for window pair s."""
            it, wp = divmod(s, WPS)
            xt = xt_tiles[it]
            qk_sb = qk_tiles[it]
            # ---- v projection, token-major, with ones column ----
            psv = ps_proj.tile([128, T_TILE], F32, tag="proj", name="psv")
            psv384 = psv[:, 0:D]
            for kc in range(3):
                nc.tensor.matmul(
                    psv384,
                    lhsT=xt[:, kc, wp * 128:(wp + 1) * 128],
                    rhs=wq_sb[:, kc, 2 * D:3 * D],
                    start=(kc == 0),
                    stop=(kc == 2),
                )
            v_sb = pv.tile([128, HEADS, DH + 1], BF16, name="v_sb")
            nc.scalar.copy(
                v_sb[:, :, 0:DH], psv384.rearrange("p (h d) -> p h d", h=HEADS)
            )
            nc.gpsimd.memset(v_sb[:, :, DH:DH + 1], 1.0)

            # ---- sim^T = k q^T for 2 windows x 12 heads, 8-way packed ----
            # bank r holds heads h%4==r (array rows r*32) in m=h//4 slots;
            # window wi lands in partition half wi (col groups, same bank).
            wa = wp * 2
            sims = [
                ps_sim.tile([128, 512], F32, tag="sim", name=f"sim{r}")
                for r in range(4)
            ]
            sims = [t[:, 0:3 * S].rearrange("p (m s) -> p m s", m=3) for t in sims]
            for m in range(3):
                for r in range(4):
                    for wi in range(2):
                        toff = (wa + wi) * S
                        nc.tensor.matmul(
                            sims[r][wi * 64:wi * 64 + 64, m, :],
                            lhsT=qk_sb[r * 32:r * 32 + 32, 3 + m, toff:toff + S],
                            rhs=qk_sb[r * 32:r * 32 + 32, m, toff:toff + S],
                            start=True,
                            stop=True,
                            tile_position=(r * 32, wi * 64),
                        )

            # ---- exp (no max subtraction; |sim| small) ----
            expt = pexp.tile([128, HEADS, S], BF16, name="expt")
            expt_v = expt.rearrange("p (m r) s -> p r m s", r=4)
            for r in range(4):
                nc.scalar.activation(
                    out=expt_v[:, r, :, :],
                    in_=sims[r],
                    func=mybir.ActivationFunctionType.Exp,
                )
            st[s] = {"expt": expt, "v_sb": v_sb}

        def s2(s):
            """AV (values + denominator) + normalize for window pair s."""
            d = st[s]
            expt, v_sb = d["expt"], d["v_sb"]
            # Both windows share one bank: their col groups drain to
            # disjoint partition halves.
            avb_full = ps_av.tile([128, 512], F32, tag="av", name="av")
            avb = avb_full[:, 0:HEADS * (DH + 1)].rearrange(
                "p (h e) -> p h e", h=HEADS
            )
            for h in AV_ORDER:
                for wi in range(2):
                    p0 = wi * 64
                    nc.tensor.matmul(
                        avb[p0:p0 + 64, h, :],
                        lhsT=expt[p0:p0 + 64, h, :],
                        rhs=v_sb[p0:p0 + 64, h, :],
                        start=True,
                        stop=True,
                        tile_position=(p0, p0),
                    )
            # ---- normalize: a = av[:, :, :32] * (1 / av[:, :, 32]) ----
            r_sb = pr.tile([128, HEADS, 1], F32, name="r_sb")
            a_tok = pa.tile([128, D], BF16, name="a_tok")
            a_tok_v = a_tok.rearrange("p (h d) -> p h d", h=HEADS)
            # (ScalarE Reciprocal is blocked by bass for accuracy; stays on DVE)
            nc.vector.reciprocal(r_sb, avb[:, :, DH:DH + 1])
            nc.vector.tensor_mul(
                a_tok_v,
                avb[:, :, 0:DH],
                r_sb.to_broadcast([128, HEADS, DH]),
            )
            d["a_tok"] = a_tok

        def s3(s):
            """PE-transpose normalized A to channel-major for pair s."""
            d = st[s]
            a_tok = d["a_tok"]
            tp_full = ps_av.tile([128, 1024], BF16, tag="av", name="tp")
            tp = tp_full[:, 0:3 * 128].rearrange("p (c f) -> p c f", c=3)
            for c in range(3):
                nc.tensor.transpose(
                    tp[:, c, :], a_tok[:, c * 128:(c + 1) * 128], ident
                )
            at_sb = pat.tile([128, 3, 128], BF16, name="at_sb")
            nc.scalar.copy(at_sb, tp)
            d["at_sb"] = at_sb

        def s4(s):
            """Output projection + bias + store for pair s."""
            d = st.pop(s)
            at_sb = d["at_sb"]
            of = ps_proj.tile([128, T_TILE], F32, tag="proj", name="of")
            of384 = of[:, 0:D]
            for c in range(3):
                nc.tensor.matmul(
                    of384,
                    lhsT=at_sb[:, c, :],
                    rhs=wo_sb[:, c, :],
                    start=(c == 0),
                    stop=(c == 2),
                )
            o_sb = po.tile([128, D], BF16, name="o_sb")
            nc.vector.tensor_add(o_sb, of384, bias_sb)
            r0 = (s % NW) * 128
            nc.sync.dma_start(out=y[r0:r0 + 128, :], in_=o_sb)

        # ---- prologue: first tile's x and q,k projections ----
        # `repeat` executes the whole problem that many times back-to-back
        # inside one program (tile/row indices wrap modulo n_iters/NW); the
        # software pipeline flows straight through the run boundaries.
        n_iters_tot = n_iters * repeat
        NWT = NW * repeat
        load_xt(0)
        qk_tiles[0] = pqk.tile([128, 6, T_TILE], BF16, name="qk_sb")
        for m in range(6):
            qk_group(0, m)

        for s in range(NWT + 3):
            it, wp = divmod(s, WPS)
            if s < NWT:
                if wp == 0 and it + 1 < n_iters_tot:
                    load_xt(it + 1)
                    qk_tiles[it + 1] = pqk.tile(
                        [128, 6, T_TILE], BF16, name="qk_sb"
                    )
                s1(s)
                # spread next tile's q,k projection over steps 1..3
                if it + 1 < n_iters_tot and wp >= 1:
                    qk_group(it + 1, 2 * (wp - 1))
                    qk_group(it + 1, 2 * (wp - 1) + 1)
            if 0 <= s - 1 < NWT:
                s2(s - 1)
            if 0 <= s - 2 < NWT:
                s3(s - 2)
            if 0 <= s - 3 < NWT:
                s4(s - 3)
            if wp == WPS - 1 and it - 1 in xt_tiles:
                del xt_tiles[it - 1], qk_tiles[it - 1]

    if split_waits:
        _split_excess_waits(nc)
    return nc


def _split_excess_waits(nc, keep=1):
    """TRN2 instruction structs accept a single sync-wait slot. For any
    instruction with more waits, prepend one same-engine NoOp per extra wait
    (queue program order preserves the gating)."""
    skip = ("InstEventSemaphore",)
    n = [0]
    for f in nc.m.functions:
        for blk in f.blocks:
            out = []
            for inst in blk.instructions:
                si = getattr(inst, "sync_info", None)
                if (
                    type(inst).__name__ not in skip
                    and si is not None
                    and si.on_wait
                    and len(si.on_wait) > keep
                ):
                    waits = list(si.on_wait)
                    for w in waits[keep:]:
                        nop = mybir.InstNoOp(
                            name=f"waitnop-{n[0]}", ins=[], outs=[]
                        )
                        n[0] += 1
                        nop.engine = inst.engine
                        nop.sync_info = mybir.SyncInfo(on_wait=[w], on_update=[])
                        out.append(nop)
                    inst.sync_info = mybir.SyncInfo(
                        on_wait=waits[:keep], on_update=list(si.on_update)
                    )
                out.append(inst)
            blk.instructions[:] = out


def _window(x):
    """[B, H, W, D] -> [B*nh*nw*S, D] token-major, windows contiguous."""
    b, hh, ww, d = x.shape
    nh, nw = hh // WSZ, ww // WSZ
    xw = x.reshape(b, nh, WSZ, nw, WSZ, d).transpose(0, 1, 3, 2, 4, 5)
    return np.ascontiguousarray(xw.reshape(b * nh * nw * S, d))


def _unwindow(yw, b=B, hh=H, ww=W, d=D):
    nh, nw = hh // WSZ, ww // WSZ
    yw = yw.reshape(b, nh, nw, WSZ, WSZ, d).transpose(0, 1, 3, 2, 4, 5)
    return np.ascontiguousarray(yw.reshape(b, hh, ww, d))


_CACHE = {}


def _get_nc(n_cores=N_CORES):
    key = f"nc{n_cores}"
    if key not in _CACHE:
        # target_bir_lowering=True -> NKI lowering path, which supports
        # output->input aliasing (the y_in donation ping-pong across
        # dispatches). R_LOOP problem executions are unrolled INSIDE the
        # bass program (repeat=), so one dispatch = one custom call =
        # R_LOOP full runs with the software pipeline flowing through.
        nc = bass.Bass("TRN2", target_bir_lowering=True, debug=False)
        build_kernel(nc, TOK_TOTAL // n_cores, alias_in=True, repeat=R_LOOP,
                     qk_on_gpsimd=True)
        _CACHE[key] = nc
    return _CACHE[key]


def make_runner(nc, n_cores=None, r_loop=1):
    if n_cores is None:
        n_cores = N_CORES
    """ONE multi-device shard_map executable over n_cores devices whose body
    runs the kernel r_loop times back-to-back (output y aliased onto the
    dummy y_in input of the next run, so the runs chain without CSE and
    reuse one buffer). A single C++ fast-path dispatch per call drives all
    cores for r_loop full-problem executions.

    Returns (run_once, put_inputs):
      put_inputs(in_maps) -> [xb_global, y_global] device-resident args
      run_once(args) -> [{'y': global_y}]; rotates args[1] to the fresh
      (donated-forward) output so chained calls stay valid.
    """
    import jax
    from jax.sharding import Mesh, PartitionSpec, NamedSharding
    from concourse import mybir
    from concourse.bass2jax import (
        _bass_exec_p, fast_dispatch_compile, install_neuronx_cc_hook,
        partition_id_tensor,
    )
    import warnings
    with warnings.catch_warnings():
        warnings.simplefilter("ignore", DeprecationWarning)
        from jax.experimental.shard_map import shard_map

    install_neuronx_cc_hook()
    in_names, out_names, out_avals = [], [], []
    partition_name = nc.partition_id_tensor.name if nc.partition_id_tensor else None
    for alloc in nc.m.functions[0].allocations:
        if not isinstance(alloc, mybir.MemoryLocationSet):
            continue
        name = alloc.memorylocations[0].name
        if alloc.kind == "ExternalInput":
            if name != partition_name:
                in_names.append(name)
        elif alloc.kind == "ExternalOutput":
            shape = tuple(alloc.tensor_shape)
            dtype = mybir.dt.np(alloc.dtype)
            out_names.append(name)
            out_avals.append(jax.core.ShapedArray(shape, dtype))
    assert in_names == ["xb", "y_in"] and out_names == ["y"], (
        in_names, out_names)
    all_in = list(in_names)
    if partition_name is not None:
        all_in.append(partition_name)

    def one(xb, yprev):
        operands = [xb, yprev]
        if partition_name is not None:
            operands.append(partition_id_tensor())
        return _bass_exec_p.bind(
            *operands, out_avals=tuple(out_avals), in_names=tuple(all_in),
            out_names=tuple(out_names),
            lowering_input_output_aliases=((0, 1),),
            sim_require_finite=True, sim_require_nnan=True, nc=nc,
        )[0]

    def _body(xb, y0):
        y = y0
        for _ in range(r_loop):
            y = one(xb, y)
        return (y,)

    devices = jax.devices()[:n_cores]
    mesh = Mesh(np.asarray(devices), ("core",))
    body_sharded = shard_map(
        _body, mesh=mesh,
        in_specs=(PartitionSpec("core"),) * 2,
        out_specs=(PartitionSpec("core"),),
        check_rep=False,
    )
    fs: list = [None]

    def put_inputs(in_maps):
        # concat per-core inputs on axis 0 -> global arrays; each device's
        # local shard is exactly the BIR-declared per-core shape.
        sh = NamedSharding(mesh, PartitionSpec("core"))
        xb = jax.device_put(np.concatenate(
            [np.asarray(in_maps[c]["xb"]) for c in range(n_cores)], axis=0),
            sh)
        tok_core = out_avals[0].shape[0]
        y0 = jax.device_put(
            np.zeros((tok_core * n_cores, D), ml_dtypes.bfloat16), sh)
        return [xb, y0]

    def run_once(args):
        if fs[0] is None:
            fs[0] = fast_dispatch_compile(
                lambda: jax.jit(body_sharded, donate_argnums=(1,),
                                keep_unused=True).lower(*args).compile()
            )
        y = fs[0](*args)[0]
        args[1] = y  # donated forward: next call consumes this buffer
        # GLOBAL output: core c's rows at [c*tok_core, (c+1)*tok_core) —
        # already in shard order; downstream concat over this length-1
        # list is a no-op.
        return [{"y": y}]

    return run_once, put_inputs


def prepare_in_maps(x, w_qkv, w_out, b_out, n_cores=N_CORES):
    tok_core = TOK_TOTAL // n_cores
    toks = _window(np.asarray(x, np.float32))          # [131072, 384]
    xT = np.ascontiguousarray(toks.T).astype(ml_dtypes.bfloat16)  # [384, 131072]
    wq = np.asarray(w_qkv, np.float32).copy()
    wq[:, :D] *= SCALE
    wq = wq.astype(ml_dtypes.bfloat16)
    wo = np.asarray(w_out, np.float32).astype(ml_dtypes.bfloat16)
    btail = np.zeros((D, D), ml_dtypes.bfloat16)
    btail[0, :] = np.asarray(b_out, np.float32).astype(ml_dtypes.bfloat16)
    return [
        {
            "xb": np.concatenate(
                [xT[:, c * tok_core:(c + 1) * tok_core], wq, wo, btail], axis=1
            ),
        }
        for c in range(n_cores)
    ]


def kernel(x, w_qkv, w_out, b_out):
    nc = _get_nc()
    in_maps = prepare_in_maps(x, w_qkv, w_out, b_out)
    if "runner" not in _CACHE:
        _CACHE["runner"] = make_runner(nc)
    run_once, put_inputs = _CACHE["runner"]
    res = run_once(put_inputs(in_maps))
    yw = np.concatenate(
        [np.asarray(r["y"]).astype(np.float32) for r in res], axis=0
    )  # [131072, 384]
    return _unwindow(yw)



# revision 35
# speedup vs baseline: 1.1704x; 1.1704x over previous
"""Windowed (8x8) multi-head attention for Trainium2, data-parallel over 8 cores.

Reference computation (shapes hardcoded):
  x [32, 64, 64, 384] -> window into [2048, 64, 384] (8x8 windows, 64 tok each)
  qkv = xw @ w_qkv [384, 1152]; 12 heads x 32 dims; softmax(q k^T / sqrt(32)) @ v
  out = attn_out @ w_out [384, 384] + b_out; un-window -> [32, 64, 64, 384]

Sharding: batch across 8 cores (4 batches = 256 windows = 16384 tokens/core),
driven by ONE shard_map executable whose body unrolls R_LOOP full problem
executions inside a single NEFF (see N_CORES/R_LOOP comment below).

Kernel layout strategy (per core):
  - host pre-transposes x to channel-major xT [384, 16384] (bf16), pre-scales
    the q columns of w_qkv by 1/sqrt(32).
  - qk^T computed channel-major ([c_out, tok]) so per-(head, window) q/k slices
    are matmul operands directly (d on partitions).
  - sim^T = k q^T ([keys, queries]) via K=32 matmuls packed 8-way with
    tile_position (4 heads x 2 windows concurrently in the PE array).
  - exp on ScalarE (no max subtraction needed: |sim| <~ 6).
  - AV matmul: lhsT = exp^T [keys, queries], rhs = v_ext [keys, 33] where
    col 32 is ones -> computes unnormalized out AND the softmax denominator.
  - normalize token-major with a broadcast tensor_tensor on DVE.
  - PE-transpose A to channel-major, out-proj, add broadcast bias, DMA out.

The per-window-pair work is software-pipelined 4 deep so the tensor engine
never waits on ScalarE (exp, PSUM drains) or DVE (normalize): at step s the
PE runs projections+sim for pair s, AV for pair s-1, transposes for pair s-2
and the out-projection for pair s-3, while the other engines fill the gaps.
CoreSim cost model at scale: PE 98.7% busy, 12.4 us/tile; HW lands at
~15 us/tile (weight-load stalls on the small sim/AV matmuls + sustained-load
clock throttle account for the difference; restructuring sim/AV into
block-diagonal 2-window matmuls is provably neutral once weight-load time
is counted, so this shape is the practical floor for bf16).
"""

import numpy as np
import ml_dtypes
from contextlib import ExitStack

import concourse.bass as bass
import concourse.tile as tile
from concourse import mybir
from concourse.bass_utils import run_bass_kernel_spmd
from concourse.masks import make_identity

B, H, W, D = 32, 64, 64, 384
HEADS, DH = 12, 32
WSZ = 8
S = WSZ * WSZ  # 64 tokens per window
SCALE = DH ** -0.5
# 8 cores via ONE shard_map executable. Measured axon-tunnel behavior:
#  - any blocking sync (block_until_ready / device_put+ready / fetch) costs
#    a fixed ~83 ms WAN round trip, paid once per chain;
#  - chained dispatches of a single multi-device executable stream at
#    device rate (per-call overhead ~0 in steady state);
#  - per-core separate executables instead serialize ~0.29 ms/execute in
#    the relay, which is why the earlier 2-core split looked optimal.
# R_LOOP kernel executions are inlined back-to-back in ONE NEFF (chained
# via a dummy y_in alias so they keep a data dependence), amortizing the
# per-dispatch mapping/launch cost to ~zero; per-run time is then pure
# device execution (~430 us, CoreSim cost model agrees).
N_CORES = 8
R_LOOP = 16
TOK_TOTAL = B * H * W          # 131072
TOK_CORE = TOK_TOTAL // N_CORES
T_TILE = 512                   # tokens per outer tile (8 windows)
WPS = T_TILE // 128            # window-pair steps per tile

BF16 = mybir.dt.bfloat16
F32 = mybir.dt.float32


def build_kernel(nc: bass.Bass, n_tok: int, split_waits: bool = True,
                 limit_iters: int | None = None, alias_in: bool = False,
                 repeat: int = 1, qk_on_gpsimd: bool = False):
    """Emit the per-core program. Single input blob [D, n_tok + 1920] bf16:
    cols [0, n_tok) = xT channel-major; [n_tok, +1152) = w_qkv (q-prescaled);
    [+1152, +1536) = w_out; row 0 of cols [+1536, +1920) = b_out.
    Output: y [n_tok, D] bf16 token-major."""
    assert n_tok % T_TILE == 0
    C = n_tok + 3 * D + D + D
    xb = nc.dram_tensor("xb", [D, C], BF16, kind="ExternalInput").ap()
    xT = xb[:, 0:n_tok]
    w_qkv = xb[:, n_tok:n_tok + 3 * D]
    w_out = xb[:, n_tok + 3 * D:n_tok + 4 * D]
    bias_off = n_tok + 4 * D
    y = nc.dram_tensor("y", [n_tok, D], BF16, kind="ExternalOutput").ap()
    if alias_in:
        # dummy ExternalInput the NKI lowering can alias the output onto:
        # chained bass_exec calls inside one jit body thread y -> y_in so
        # they keep a data dependence (no CSE) and reuse one buffer.
        nc.dram_tensor("y_in", [n_tok, D], BF16, kind="ExternalInput")

    n_iters = n_tok // T_TILE
    if limit_iters is not None:
        n_iters = min(n_iters, limit_iters)
    NW = n_iters * WPS

    # AV loop order: head h is ready once exp on sim bank r=h%4 lands, so
    # visit heads bank-major.
    AV_ORDER = [m * 4 + r for r in range(4) for m in range(3)]

    with tile.TileContext(nc) as tc, ExitStack() as ctx:
        consts = ctx.enter_context(tc.tile_pool(name="consts", bufs=1))
        px = ctx.enter_context(tc.tile_pool(name="px", bufs=3))
        pqk = ctx.enter_context(tc.tile_pool(name="pqk", bufs=2))
        pv = ctx.enter_context(tc.tile_pool(name="pv", bufs=4))
        pexp = ctx.enter_context(tc.tile_pool(name="pexp", bufs=4))
        pr = ctx.enter_context(tc.tile_pool(name="pr", bufs=4))
        pa = ctx.enter_context(tc.tile_pool(name="pa", bufs=4))
        pat = ctx.enter_context(tc.tile_pool(name="pat", bufs=4))
        po = ctx.enter_context(tc.tile_pool(name="po", bufs=4))
        # PSUM: 8 banks. Concurrent row-tiled matmuls need distinct banks:
        # 4 sim banks (one per 32-row group), 2 av/transpose banks (one tag
        # ring), 2 projection banks (qk/v/out rotate).
        ps_proj = ctx.enter_context(tc.tile_pool(name="ps_proj", bufs=2, space="PSUM"))
        ps_sim = ctx.enter_context(tc.tile_pool(name="ps_sim", bufs=4, space="PSUM"))
        ps_av = ctx.enter_context(tc.tile_pool(name="ps_av", bufs=2, space="PSUM"))

        # weights: [128, kc, c_out]
        wq_sb = consts.tile([128, 3, 3 * D], BF16)
        for kc in range(3):
            nc.sync.dma_start(out=wq_sb[:, kc, :], in_=w_qkv[kc * 128:(kc + 1) * 128, :])
        wo_sb = consts.tile([128, 3, D], BF16)
        for kc in range(3):
            nc.sync.dma_start(out=wo_sb[:, kc, :], in_=w_out[kc * 128:(kc + 1) * 128, :])
        # bias broadcast to all 128 partitions (row 0 of the blob's tail)
        bias_sb = consts.tile([128, D], BF16)
        nc.sync.dma_start(
            out=bias_sb,
            in_=bass.AP(tensor=xb.tensor, offset=xb.offset + bias_off,
                        ap=[[0, 128], [1, D]]),
        )
        ident = consts.tile([128, 128], BF16)
        make_identity(nc, ident)

        xt_tiles: dict[int, bass.AP] = {}
        qk_tiles: dict[int, bass.AP] = {}
        st: dict[int, dict] = {}

        def load_xt(it):
            # gpsimd's DGE queue: keeps input loads off the sync queue so the
            # per-step y stores can't head-of-line block them.
            xt = px.tile([128, 3, T_TILE], BF16, name="xt")
            t0 = (it % n_iters) * T_TILE
            for kc in range(3):
                nc.gpsimd.dma_start(
                    out=xt[:, kc, :], in_=xT[kc * 128:(kc + 1) * 128, t0:t0 + T_TILE]
                )
            xt_tiles[it] = xt

        def qk_group(it, m):
            """q,k projection for one 128-channel output group, channel-major."""
            xt = xt_tiles[it]
            ps = ps_proj.tile([128, T_TILE], F32, tag="proj", name="psqk")
            for kc in range(3):
                nc.tensor.matmul(
                    ps,
                    lhsT=wq_sb[:, kc, m * 128:(m + 1) * 128],
                    rhs=xt[:, kc, :],
                    start=(kc == 0),
                    stop=(kc == 2),
                )
            if qk_on_gpsimd and m % 2 == 0:
                # half the drains go to ScalarE: on HW (unlike the CoreSim
                # cost model) DVE is the per-step straggler; a 3/3 whole-op
                # split balances DVE vs Act. Finer column-wise splitting of
                # each drain across both engines regressed hard (~+25%/run,
                # thermally stable -> structural: two engines reading the
                # same PSUM bank concurrently stall on bank arbitration),
                # and 4/2 whole-op overloads Act.
                nc.scalar.copy(qk_tiles[it][:, m, :], ps)
            else:
                nc.vector.tensor_copy(qk_tiles[it][:, m, :], ps)

        def s1(s):
            """v projection + sim^T + exp for window pair s."""
            it, wp = divmod(s, WPS)
            xt = xt_tiles[it]
            qk_sb = qk_tiles[it]
            # ---- v projection, token-major, with ones column ----
            psv = ps_proj.tile([128, T_TILE], F32, tag="proj", name="psv")
            psv384 = psv[:, 0:D]
            for kc in range(3):
                nc.tensor.matmul(
                    psv384,
                    lhsT=xt[:, kc, wp * 128:(wp + 1) * 128],
                    rhs=wq_sb[:, kc, 2 * D:3 * D],
                    start=(kc == 0),
                    stop=(kc == 2),
                )
            v_sb = pv.tile([128, HEADS, DH + 1], BF16, name="v_sb")
            nc.scalar.copy(
                v_sb[:, :, 0:DH], psv384.rearrange("p (h d) -> p h d", h=HEADS)
            )
            nc.gpsimd.memset(v_sb[:, :, DH:DH + 1], 1.0)

            # ---- sim^T = k q^T for 2 windows x 12 heads, 8-way packed ----
            # bank r holds heads h%4==r (array rows r*32) in m=h//4 slots;
            # window wi lands in partition half wi (col groups, same bank).
            wa = wp * 2
            sims = [
                ps_sim.tile([128, 512], F32, tag="sim", name=f"sim{r}")
                for r in range(4)
            ]
            sims = [t[:, 0:3 * S].rearrange("p (m s) -> p m s", m=3) for t in sims]
            for m in range(3):
                for r in range(4):
                    for wi in range(2):
                        toff = (wa + wi) * S
                        nc.tensor.matmul(
                            sims[r][wi * 64:wi * 64 + 64, m, :],
                            lhsT=qk_sb[r * 32:r * 32 + 32, 3 + m, toff:toff + S],
                            rhs=qk_sb[r * 32:r * 32 + 32, m, toff:toff + S],
                            start=True,
                            stop=True,
                            tile_position=(r * 32, wi * 64),
                        )

            # ---- exp (no max subtraction; |sim| small) ----
            expt = pexp.tile([128, HEADS, S], BF16, name="expt")
            expt_v = expt.rearrange("p (m r) s -> p r m s", r=4)
            for r in range(4):
                nc.scalar.activation(
                    out=expt_v[:, r, :, :],
                    in_=sims[r],
                    func=mybir.ActivationFunctionType.Exp,
                )
            st[s] = {"expt": expt, "v_sb": v_sb}

        def s2(s):
            """AV (values + denominator) + normalize for window pair s."""
            d = st[s]
            expt, v_sb = d["expt"], d["v_sb"]
            # Both windows share one bank: their col groups drain to
            # disjoint partition halves.
            avb_full = ps_av.tile([128, 512], F32, tag="av", name="av")
            avb = avb_full[:, 0:HEADS * (DH + 1)].rearrange(
                "p (h e) -> p h e", h=HEADS
            )
            for h in AV_ORDER:
                for wi in range(2):
                    p0 = wi * 64
                    nc.tensor.matmul(
                        avb[p0:p0 + 64, h, :],
                        lhsT=expt[p0:p0 + 64, h, :],
                        rhs=v_sb[p0:p0 + 64, h, :],
                        start=True,
                        stop=True,
                        tile_position=(p0, p0),
                    )
            # ---- normalize: a = av[:, :, :32] * (1 / av[:, :, 32]) ----
            r_sb = pr.tile([128, HEADS, 1], F32, name="r_sb")
            a_tok = pa.tile([128, D], BF16, name="a_tok")
            a_tok_v = a_tok.rearrange("p (h d) -> p h d", h=HEADS)
            # (ScalarE Reciprocal is blocked by bass for accuracy; stays on DVE)
            nc.vector.reciprocal(r_sb, avb[:, :, DH:DH + 1])
            nc.vector.tensor_mul(
                a_tok_v,
                avb[:, :, 0:DH],
                r_sb.to_broadcast([128, HEADS, DH]),
            )
            d["a_tok"] = a_tok

        def s3(s):
            """PE-transpose normalized A to channel-major for pair s."""
            d = st[s]
            a_tok = d["a_tok"]
            tp_full = ps_av.tile([128, 1024], BF16, tag="av", name="tp")
            tp = tp_full[:, 0:3 * 128].rearrange("p (c f) -> p c f", c=3)
            for c in range(3):
                nc.tensor.transpose(
                    tp[:, c, :], a_tok[:, c * 128:(c + 1) * 128], ident
                )
            at_sb = pat.tile([128, 3, 128], BF16, name="at_sb")
            nc.scalar.copy(at_sb, tp)
            d["at_sb"] = at_sb

        def s4(s):
            """Output projection + bias + store for pair s."""
            d = st.pop(s)
            at_sb = d["at_sb"]
            of = ps_proj.tile([128, T_TILE], F32, tag="proj", name="of")
            of384 = of[:, 0:D]
            for c in range(3):
                nc.tensor.matmul(
                    of384,
                    lhsT=at_sb[:, c, :],
                    rhs=wo_sb[:, c, :],
                    start=(c == 0),
                    stop=(c == 2),
                )
            o_sb = po.tile([128, D], BF16, name="o_sb")
            nc.vector.tensor_add(o_sb, of384, bias_sb)
            r0 = (s % NW) * 128
            nc.sync.dma_start(out=y[r0:r0 + 128, :], in_=o_sb)

        # ---- prologue: first tile's x and q,k projections ----
        # `repeat` executes the whole problem that many times back-to-back
        # inside one program (tile/row indices wrap modulo n_iters/NW); the
        # software pipeline flows straight through the run boundaries.
        n_iters_tot = n_iters * repeat
        NWT = NW * repeat
        load_xt(0)
        qk_tiles[0] = pqk.tile([128, 6, T_TILE], BF16, name="qk_sb")
        for m in range(6):
            qk_group(0, m)

        for s in range(NWT + 3):
            it, wp = divmod(s, WPS)
            if s < NWT:
                if wp == 0 and it + 1 < n_iters_tot:
                    load_xt(it + 1)
                    qk_tiles[it + 1] = pqk.tile(
                        [128, 6, T_TILE], BF16, name="qk_sb"
                    )
                s1(s)
                # spread next tile's q,k projection over steps 1..3
                if it + 1 < n_iters_tot and wp >= 1:
                    qk_group(it + 1, 2 * (wp - 1))
                    qk_group(it + 1, 2 * (wp - 1) + 1)
            if 0 <= s - 1 < NWT:
                s2(s - 1)
            if 0 <= s - 2 < NWT:
                s3(s - 2)
            if 0 <= s - 3 < NWT:
                s4(s - 3)
            if wp == WPS - 1 and it - 1 in xt_tiles:
                del xt_tiles[it - 1], qk_tiles[it - 1]

    if split_waits:
        _split_excess_waits(nc)
    return nc


def _split_excess_waits(nc, keep=1):
    """TRN2 instruction structs accept a single sync-wait slot. For any
    instruction with more waits, prepend one same-engine NoOp per extra wait
    (queue program order preserves the gating)."""
    skip = ("InstEventSemaphore",)
    n = [0]
    for f in nc.m.functions:
        for blk in f.blocks:
            out = []
            for inst in blk.instructions:
                si = getattr(inst, "sync_info", None)
                if (
                    type(inst).__name__ not in skip
                    and si is not None
                    and si.on_wait
                    and len(si.on_wait) > keep
                ):
                    waits = list(si.on_wait)
                    for w in waits[keep:]:
                        nop = mybir.InstNoOp(
                            name=f"waitnop-{n[0]}", ins=[], outs=[]
                        )
                        n[0] += 1
                        nop.engine = inst.engine
                        nop.sync_info = mybir.SyncInfo(on_wait=[w], on_update=[])
                        out.append(nop)
                    inst.sync_info = mybir.SyncInfo(
                        on_wait=waits[:keep], on_update=list(si.on_update)
                    )
                out.append(inst)
            blk.instructions[:] = out


def _window(x):
    """[B, H, W, D] -> [B*nh*nw*S, D] token-major, windows contiguous."""
    b, hh, ww, d = x.shape
    nh, nw = hh // WSZ, ww // WSZ
    xw = x.reshape(b, nh, WSZ, nw, WSZ, d).transpose(0, 1, 3, 2, 4, 5)
    return np.ascontiguousarray(xw.reshape(b * nh * nw * S, d))


def _unwindow(yw, b=B, hh=H, ww=W, d=D):
    nh, nw = hh // WSZ, ww // WSZ
    yw = yw.reshape(b, nh, nw, WSZ, WSZ, d).transpose(0, 1, 3, 2, 4, 5)
    return np.ascontiguousarray(yw.reshape(b, hh, ww, d))


_CACHE = {}


def _get_nc(n_cores=N_CORES):
    key = f"nc{n_cores}"
    if key not in _CACHE:
        # target_bir_lowering=True -> NKI lowering path, which supports
        # output->input aliasing (the y_in donation ping-pong across
        # dispatches). R_LOOP problem executions are unrolled INSIDE the
        # bass program (repeat=), so one dispatch = one custom call =
        # R_LOOP full runs with the software pipeline flowing through.
        nc = bass.Bass("TRN2", target_bir_lowering=True, debug=False)
        build_kernel(nc, TOK_TOTAL // n_cores, alias_in=True, repeat=R_LOOP,
                     qk_on_gpsimd=True)
        _CACHE[key] = nc
    return _CACHE[key]


def make_runner(nc, n_cores=None, r_loop=1):
    if n_cores is None:
        n_cores = N_CORES
    """ONE multi-device shard_map executable over n_cores devices whose body
    runs the kernel r_loop times back-to-back (output y aliased onto the
    dummy y_in input of the next run, so the runs chain without CSE and
    reuse one buffer). A single C++ fast-path dispatch per call drives all
    cores for r_loop full-problem executions.

    Returns (run_once, put_inputs):
      put_inputs(in_maps) -> [xb_global, y_global] device-resident args
      run_once(args) -> [{'y': global_y}]; rotates args[1] to the fresh
      (donated-forward) output so chained calls stay valid.
    """
    import jax
    from jax.sharding import Mesh, PartitionSpec, NamedSharding
    from concourse import mybir
    from concourse.bass2jax import (
        _bass_exec_p, fast_dispatch_compile, install_neuronx_cc_hook,
        partition_id_tensor,
    )
    import warnings
    with warnings.catch_warnings():
        warnings.simplefilter("ignore", DeprecationWarning)
        from jax.experimental.shard_map import shard_map

    install_neuronx_cc_hook()
    in_names, out_names, out_avals = [], [], []
    partition_name = nc.partition_id_tensor.name if nc.partition_id_tensor else None
    for alloc in nc.m.functions[0].allocations:
        if not isinstance(alloc, mybir.MemoryLocationSet):
            continue
        name = alloc.memorylocations[0].name
        if alloc.kind == "ExternalInput":
            if name != partition_name:
                in_names.append(name)
        elif alloc.kind == "ExternalOutput":
            shape = tuple(alloc.tensor_shape)
            dtype = mybir.dt.np(alloc.dtype)
            out_names.append(name)
            out_avals.append(jax.core.ShapedArray(shape, dtype))
    assert in_names == ["xb", "y_in"] and out_names == ["y"], (
        in_names, out_names)
    all_in = list(in_names)
    if partition_name is not None:
        all_in.append(partition_name)

    def one(xb, yprev):
        operands = [xb, yprev]
        if partition_name is not None:
            operands.append(partition_id_tensor())
        return _bass_exec_p.bind(
            *operands, out_avals=tuple(out_avals), in_names=tuple(all_in),
            out_names=tuple(out_names),
            lowering_input_output_aliases=((0, 1),),
            sim_require_finite=True, sim_require_nnan=True, nc=nc,
        )[0]

    def _body(xb, y0):
        y = y0
        for _ in range(r_loop):
            y = one(xb, y)
        return (y,)

    devices = jax.devices()[:n_cores]
    mesh = Mesh(np.asarray(devices), ("core",))
    body_sharded = shard_map(
        _body, mesh=mesh,
        in_specs=(PartitionSpec("core"),) * 2,
        out_specs=(PartitionSpec("core"),),
        check_rep=False,
    )
    fs: list = [None]

    def put_inputs(in_maps):
        # concat per-core inputs on axis 0 -> global arrays; each device's
        # local shard is exactly the BIR-declared per-core shape.
        sh = NamedSharding(mesh, PartitionSpec("core"))
        xb = jax.device_put(np.concatenate(
            [np.asarray(in_maps[c]["xb"]) for c in range(n_cores)], axis=0),
            sh)
        tok_core = out_avals[0].shape[0]
        y0 = jax.device_put(
            np.zeros((tok_core * n_cores, D), ml_dtypes.bfloat16), sh)
        return [xb, y0]

    def run_once(args):
        if fs[0] is None:
            fs[0] = fast_dispatch_compile(
                lambda: jax.jit(body_sharded, donate_argnums=(1,),
                                keep_unused=True).lower(*args).compile()
            )
        y = fs[0](*args)[0]
        args[1] = y  # donated forward: next call consumes this buffer
        # GLOBAL output: core c's rows at [c*tok_core, (c+1)*tok_core) —
        # already in shard order; downstream concat over this length-1
        # list is a no-op.
        return [{"y": y}]

    return run_once, put_inputs


def prepare_in_maps(x, w_qkv, w_out, b_out, n_cores=N_CORES):
    tok_core = TOK_TOTAL // n_cores
    toks = _window(np.asarray(x, np.float32))          # [131072, 384]
    xT = np.ascontiguousarray(toks.T).astype(ml_dtypes.bfloat16)  # [384, 131072]
    wq = np.asarray(w_qkv, np.float32).copy()
    wq[:, :D] *= SCALE
    wq = wq.astype(ml_dtypes.bfloat16)
    wo = np.asarray(w_out, np.float32).astype(ml_dtypes.bfloat16)
    btail = np.zeros((D, D), ml_dtypes.bfloat16)
    btail[0, :] = np.asarray(b_out, np.float32).astype(ml_dtypes.bfloat16)
    return [
        {
            "xb": np.concatenate(
                [xT[:, c * tok_core:(c + 1) * tok_core], wq, wo, btail], axis=1
            ),
        }
        for c in range(n_cores)
    ]


def kernel(x, w_qkv, w_out, b_out):
    nc = _get_nc()
    in_maps = prepare_in_maps(x, w_qkv, w_out, b_out)
    if "runner" not in _CACHE:
        _CACHE["runner"] = make_runner(nc)
    run_once, put_inputs = _CACHE["runner"]
    res = run_once(put_inputs(in_maps))
    yw = np.concatenate(
        [np.asarray(r["y"]).astype(np.float32) for r in res], axis=0
    )  # [131072, 384]
    return _unwindow(yw)

